# revision 24
# baseline (speedup 1.0000x reference)
"""AttentionBlock (GroupNorm -> qkv conv1x1 -> 4-head attention -> proj + residual)
on 8 Trainium2 NeuronCores.

Sharding: B*NH = 2*4 = 8 (batch, head) pairs -> one per core.

Host folds everything input-dependent that is O(C*N): GroupNorm stats
(mu, rstd per group), gamma/beta, qk scale s1 and the alpha q/k range
split are all folded into the shipped fp8 weights W'' and exact f32
biases.  x ships as fp8e4 in kt-pair layout.

Per core (head h of batch b):
  - qkv GEMM in fp8 DoubleRow (contraction 256/pass), bias added during
    the PSUM->SBUF copies on DVE; v transposed on the PE (identity
    matmul) and packed to fp8 for DoubleRow.
  - scoresT[s,t] = k_tile.T @ q (bf16, s on partitions)
  - exp on ACT only: fp8e4 out with +2.8 offset (cancels in partial/Z)
  - attn@v: fp8 DoubleRow over s-tile pairs
  - Z[t]: most pairs summed by DoubleRow ones-matmuls on PE, the rest by
    a DVE fp16 add tree joined into the same PSUM group (round 3 uses
    the earliest pairs for the tree so the drain is not gated on it).
  - proj on PE (bf16); partial shipped bf16; v-bias compensated on host
    (attention rows sum to 1, so it shifts h/Z by exactly bias_v).
Host: out[b] = x + b_proj + sum_h (partial_h/Z_h + Wp_h @ bias_v_h).
"""

import math
from contextlib import ExitStack

import ml_dtypes
import numpy as np

import concourse.bacc as bacc
import concourse.bass as bass
import concourse.mybir as mybir
import concourse.tile as tile
from concourse.bass_utils import run_bass_kernel_spmd

C = 512
NH = 4
G = 32
EPS = 1e-5
N = 4096
CH = 128
B = 2
NCORES = 8
TCHUNK = 1024
NCHUNK = N // TCHUNK     # 4
NST = N // 128           # 32
NPAIR = NST // 2         # 16
N_DVE_Z = 6              # s-tile pairs of Z summed on the DVE tree
EXPB = 2.8               # exp offset: e^(score+2.8) stays within fp8e4 range
ALPHA = 4.0              # q-row fold (k rows get 1/ALPHA); cancels in q.k

F32 = mybir.dt.float32
BF16 = mybir.dt.bfloat16
F16 = mybir.dt.float16
F8 = mybir.dt.float8e4
DR = mybir.MatmulPerfMode.DoubleRow
AF = mybir.ActivationFunctionType

TRACE = False
TRACE_CORES = [0]
LAST_RESULT = None


def build_program():
    nc = bacc.Bacc()

    # x2[pr, p, i, n] = x[128*(2pr+i)+p, n]  (kt-pair layout for DoubleRow)
    x2d = nc.declare_dram_parameter("x2", [2, 128, 2, N], F8, isOutput=False)
    # wq8[pr, p, i, o] = W''[o, 128*(2pr+i)+p]  (all folds + rstd, fp8)
    wq8d = nc.declare_dram_parameter("wq8", [2, 128, 2, 384], F8, isOutput=False)
    bqkv = nc.declare_dram_parameter("bqkv", [128, 2], F32, isOutput=False)
    wprojT = nc.declare_dram_parameter("wprojT", [CH, C], BF16, isOutput=False)
    ident = nc.declare_dram_parameter("ident", [128, 128], BF16, isOutput=False)
    partial = nc.declare_dram_parameter("partial", [C, N], BF16, isOutput=True)
    zout = nc.declare_dram_parameter("zout", [1, N], F32, isOutput=True)

    with tile.TileContext(nc) as tc, ExitStack() as ctx:
        consts = ctx.enter_context(tc.tile_pool(name="consts", bufs=1))
        xpool = ctx.enter_context(tc.tile_pool(name="xpool", bufs=2))
        qkp = ctx.enter_context(tc.tile_pool(name="qkp", bufs=1))
        epool = ctx.enter_context(tc.tile_pool(name="epool", bufs=32))
        trpool = ctx.enter_context(tc.tile_pool(name="trpool", bufs=14))
        hpool = ctx.enter_context(tc.tile_pool(name="hpool", bufs=2))
        opool = ctx.enter_context(tc.tile_pool(name="opool", bufs=3))
        zsp = ctx.enter_context(tc.tile_pool(name="zsp", bufs=1))
        ps_sc = ctx.enter_context(tc.tile_pool(name="ps_sc", bufs=2, space="PSUM"))
        ps_acc = ctx.enter_context(tc.tile_pool(name="ps_acc", bufs=2, space="PSUM"))
        ps_mm2 = ctx.enter_context(tc.tile_pool(name="ps_mm2", bufs=2, space="PSUM"))

        # ---- x first (3 queues, chunk-0-relevant halves first), then weights
        xt = [xpool.tile([128, 2, N], F8, tag="xt", name=f"x{pr}")
              for pr in range(2)]
        H2 = N // 2
        w8 = []
        for pr in range(2):
            w8.append(consts.tile([128, 2, 384], F8, tag=f"w8{pr}", name=f"w8{pr}"))
        # x streamed as kt-halves: the low halves (cols 0:2048) land first so
        # the k/q chunk GEMMs and round-0 scores can start while the rest loads
        qs = [nc.sync, nc.scalar, nc.scalar, nc.sync]
        for hv in range(2):
            for kt in range(4):
                pr, i = divmod(kt, 2)
                qs[kt].dma_start(
                    out=xt[pr][:, i, hv * H2 : (hv + 1) * H2],
                    in_=x2d[pr, :, i, hv * H2 : (hv + 1) * H2],
                )
            if hv == 0:
                for pr in range(2):
                    nc.scalar.dma_start(out=w8[pr], in_=wq8d[pr])
        bq_sb = consts.tile([128, 2], F32, tag="bq")
        nc.gpsimd.dma_start(out=bq_sb, in_=bqkv[:, :])
        wp_sb = consts.tile([CH, C], BF16, tag="wp")
        nc.gpsimd.dma_start(out=wp_sb, in_=wprojT[:, :])
        ident_sb = consts.tile([128, 128], BF16, tag="ident")
        nc.gpsimd.dma_start(out=ident_sb, in_=ident[:, :])
        ones2 = consts.tile([128, 2, 16], F8, tag="ones2")
        nc.vector.memset(ones2, 1.0)
        ones1 = consts.tile([128, 1], F16, tag="ones1")
        nc.vector.memset(ones1, 1.0)
        bexp = consts.tile([128, 1], F32, tag="bexp")
        nc.vector.memset(bexp, EXPB)

        # ---- qkv chunk GEMM (DoubleRow) ----
        q_sb = qkp.tile([128, N], BF16, tag="q_sb")
        k_sb = qkp.tile([128, N], BF16, tag="k_sb")
        v_sb = qkp.tile([128, N], BF16, tag="v_sb")
        vT2 = qkp.tile([128, NST, 128], F8, tag="vT2")

        def qkv_chunk(j, ch):
            ps = ps_mm2.tile([128, 512], F32, tag="mm2", name=f"qkv{j}_{ch}")
            for pr in range(2):
                nc.tensor.matmul(
                    ps,
                    lhsT=w8[pr][:, :, 128 * j : 128 * (j + 1)],
                    rhs=xt[pr][:, :, 512 * ch : 512 * (ch + 1)],
                    start=(pr == 0),
                    stop=(pr == 1),
                    perf_mode=DR,
                )
            dst = (q_sb, k_sb, v_sb)[j]
            if j < 2:
                nc.vector.tensor_scalar(
                    out=dst[:, 512 * ch : 512 * (ch + 1)],
                    in0=ps,
                    scalar1=bq_sb[:, j : j + 1],
                    scalar2=None,
                    op0=mybir.AluOpType.add,
                )
            else:
                nc.vector.tensor_copy(
                    out=dst[:, 512 * ch : 512 * (ch + 1)], in_=ps
                )
                # transpose the 4 s-tiles of this chunk on the PE, then pack
                # bf16 psum -> fp8 vT2 in one DVE copy
                ps_tr = ps_acc.tile([128, 4, 128], BF16, tag="acc",
                                    name=f"tr{ch}")
                for l in range(4):
                    stt = 4 * ch + l
                    nc.tensor.transpose(
                        ps_tr[:, l, :],
                        v_sb[:, 128 * stt : 128 * (stt + 1)],
                        ident_sb,
                    )
                nc.vector.tensor_copy(
                    out=vT2[:, 4 * ch : 4 * (ch + 1), :], in_=ps_tr
                )

        # ---- round machinery ----
        et_all = []          # per round: list of NPAIR pair tiles
        esums = [None] * NCHUNK
        ph_all = [None] * NCHUNK

        def tree_pairs(r):
            # last round: tree over the EARLY pairs (their exps land first)
            # and more of them, so the drain isn't gated on PE z-matmuls
            return range(0, 10) if r == NCHUNK - 1 else \
                range(NPAIR - N_DVE_Z, NPAIR)

        def pe_pairs(r):
            return [p for p in range(NPAIR) if p not in set(tree_pairs(r))]

        def emit_scores(r, stt):
            ps = ps_sc.tile([128, TCHUNK], F32, tag="sc")
            ksl = k_sb[:, 128 * stt : 128 * (stt + 1)]
            t0 = r * TCHUNK
            for hh in range(2):
                nc.tensor.matmul(
                    ps[:, 512 * hh : 512 * (hh + 1)],
                    lhsT=ksl,
                    rhs=q_sb[:, t0 + 512 * hh : t0 + 512 * (hh + 1)],
                    start=True,
                    stop=True,
                )
            if stt % 2 == 0:
                et = epool.tile([128, 2, TCHUNK], F8, tag="et")
                et_all[r].append(et)
            nc.scalar.activation(
                out=et_all[r][stt // 2][:, stt % 2, :],
                in_=ps,
                func=AF.Exp,
                bias=bexp,
            )

        def emit_attnv_pair(r, p):
            ph = ph_all[r]
            for hh in range(2):
                nc.tensor.matmul(
                    ph[hh],
                    lhsT=vT2[:, 2 * p : 2 * p + 2, :],
                    rhs=et_all[r][p][:, :, 512 * hh : 512 * (hh + 1)],
                    start=(p == 0),
                    stop=(p == NPAIR - 1),
                    perf_mode=DR,
                )

        def emit_z(r):
            # both t-halves; PE pairs + fp16 esum join
            for hh in range(2):
                zps = ps_mm2.tile([1, 512], F32, tag="mm2", name=f"z{r}_{hh}")
                pep = pe_pairs(r)
                for n_, p in enumerate(pep):
                    nc.tensor.matmul(
                        zps,
                        lhsT=ones2[:, :, 0:1],
                        rhs=et_all[r][p][:, :, 512 * hh : 512 * (hh + 1)],
                        start=(n_ == 0),
                        stop=False,
                        perf_mode=DR,
                    )
                nc.tensor.matmul(
                    zps,
                    lhsT=ones1,
                    rhs=esums[r][:, 512 * hh : 512 * (hh + 1)],
                    start=False,
                    stop=True,
                )
                nc.vector.tensor_copy(
                    out=z_sb[:, r * TCHUNK + 512 * hh : r * TCHUNK + 512 * (hh + 1)],
                    in_=zps,
                )

        def emit_tree(r):
            tt = []
            for p in tree_pairs(r):
                t_ = trpool.tile([128, TCHUNK], F16, tag="tr", name=f"t{r}_{p}")
                nc.vector.tensor_add(
                    out=t_, in0=et_all[r][p][:, 0, :], in1=et_all[r][p][:, 1, :]
                )
                tt.append(t_)
            while len(tt) > 1:
                nxt = []
                for a in range(0, len(tt) - 1, 2):
                    nc.vector.tensor_add(out=tt[a], in0=tt[a], in1=tt[a + 1])
                    nxt.append(tt[a])
                if len(tt) % 2 == 1:
                    nxt.append(tt[-1])
                tt = nxt
            esums[r] = tt[0]

        def emit_hcopy_proj(r, wide=False):
            hsb = hpool.tile([128, TCHUNK], BF16, tag="h")
            for hh in range(2):
                nc.vector.tensor_copy(
                    out=hsb[:, 512 * hh : 512 * (hh + 1)], in_=ph_all[r][hh]
                )
            for ot in range(4):
                if wide:
                    # drain only: scores pool is free, use its wide slots to
                    # avoid the proj->copy ping-pong on the 2-slot mm2 pool
                    psp = ps_sc.tile([128, TCHUNK], F32, tag="sc",
                                     name=f"pjw{r}_{ot}")
                    for hh in range(2):
                        nc.tensor.matmul(
                            psp[:, 512 * hh : 512 * (hh + 1)],
                            lhsT=wp_sb[:, 128 * ot : 128 * (ot + 1)],
                            rhs=hsb[:, 512 * hh : 512 * (hh + 1)],
                            start=True,
                            stop=True,
                        )
                    ob = opool.tile([128, TCHUNK], BF16, tag="obw",
                                    name=f"obw{r}_{ot}")
                    nc.vector.tensor_copy(out=ob, in_=psp)
                    nc.sync.dma_start(
                        out=partial[
                            128 * ot : 128 * (ot + 1),
                            r * TCHUNK : (r + 1) * TCHUNK,
                        ],
                        in_=ob,
                    )
                    continue
                for hh in range(2):
                    psp = ps_mm2.tile([128, 512], F32, tag="mm2",
                                      name=f"pj{r}_{ot}_{hh}")
                    nc.tensor.matmul(
                        psp,
                        lhsT=wp_sb[:, 128 * ot : 128 * (ot + 1)],
                        rhs=hsb[:, 512 * hh : 512 * (hh + 1)],
                        start=True,
                        stop=True,
                    )
                    ob = opool.tile([128, 512], BF16, tag="ob")
                    nc.vector.tensor_copy(out=ob, in_=psp)
                    nc.sync.dma_start(
                        out=partial[
                            128 * ot : 128 * (ot + 1),
                            r * TCHUNK + 512 * hh : r * TCHUNK + 512 * (hh + 1),
                        ],
                        in_=ob,
                    )

        z_sb = zsp.tile([1, N], F32, tag="z_sb")

        # ================= round 0 =================
        et_all.append([])
        qkv_chunk(1, 0)
        qkv_chunk(0, 0)
        qkv_chunk(0, 1)
        emit_scores(0, 0)
        emit_scores(0, 1)
        fill0 = [("k", ch) for ch in range(1, 8)]
        extra0 = [("v", ch) for ch in range(8)] + [("q", 2), ("q", 3)]
        ei = 0
        for stt in range(2, NST):
            ch = stt // 4 + 1
            if fill0 and stt % 4 == 2 and ch < 8:
                qkv_chunk(1, ch)
                fill0.pop(0)
            if stt % 2 == 1 and ei < len(extra0):
                kind, ch2 = extra0[ei]
                qkv_chunk(2 if kind == "v" else 0, ch2)
                ei += 1
            emit_scores(0, stt)
        while ei < len(extra0):
            kind, ch2 = extra0[ei]
            qkv_chunk(2 if kind == "v" else 0, ch2)
            ei += 1
        emit_tree(0)

        # ================= rounds 1..3 =================
        ap3 = 0
        for r in range(1, NCHUNK):
            last = r == NCHUNK - 1
            et_all.append([])
            ph_all[r - 1] = [
                ps_acc.tile([128, 512], F32, tag="acc", name=f"h{r-1}_{hh}")
                for hh in range(2)
            ]
            if last:
                ph_all[r] = [
                    ps_acc.tile([128, 512], F32, tag="acc", name=f"h{r}_{hh}")
                    for hh in range(2)
                ]
            if r >= 2:
                emit_hcopy_proj(r - 2)
            pend_q = [("q", 4), ("q", 5), ("q", 6), ("q", 7)] if r == 1 else []
            ap = 0
            denom = 20 if last else NST
            for stt in range(NST):
                emit_scores(r, stt)
                if pend_q and stt % 2 == 1:
                    qkv_chunk(0, pend_q.pop(0)[1])
                if stt == 10:
                    emit_z(r - 1)
                want = min(NPAIR, ((stt + 1) * NPAIR) // denom)
                while ap < want:
                    emit_attnv_pair(r - 1, ap)
                    ap += 1
                if last:
                    # ph[3] reuses ph[2]'s psum slots: attnv(3) may only
                    # start after the h(2) copies, or the PE queue deadlocks
                    if stt == 20:
                        emit_hcopy_proj(r - 1)
                    while ap3 < min(max(0, stt - 21), NPAIR - 1):
                        emit_attnv_pair(r, ap3)
                        ap3 += 1
            while ap < NPAIR:
                emit_attnv_pair(r - 1, ap)
                ap += 1
            emit_tree(r)

        # ================= drain =================
        r = NCHUNK - 1
        while ap3 < NPAIR:
            emit_attnv_pair(r, ap3)
            ap3 += 1
        emit_z(r)
        emit_hcopy_proj(r, wide=True)
        nc.sync.dma_start(out=zout[:, :], in_=z_sb)

    if not nc.is_finalized():
        nc.finalize()
    return nc


_NC_CACHE = None


def _get_nc():
    global _NC_CACHE
    if _NC_CACHE is None:
        _NC_CACHE = build_program()
    return _NC_CACHE


def kernel(x, norm_w, norm_b, w_qkv, w_proj, b_proj):
    global LAST_RESULT
    x = np.asarray(x, dtype=np.float32)
    norm_w = np.asarray(norm_w, dtype=np.float32)
    norm_b = np.asarray(norm_b, dtype=np.float32)
    w_qkv = np.asarray(w_qkv, dtype=np.float32)
    w_proj = np.asarray(w_proj, dtype=np.float32)
    b_proj = np.asarray(b_proj, dtype=np.float32)

    s1 = 1.0 / math.sqrt(math.sqrt(CH))
    bf16 = ml_dtypes.bfloat16
    f8 = ml_dtypes.float8_e4m3

    # host-side GroupNorm stats (folded into W''/biases; O(C*N) prep)
    xr = x.reshape(B, G, C // G * N)
    mu_g = xr.mean(axis=2)
    var_g = xr.var(axis=2)
    rstd_g = 1.0 / np.sqrt(var_g + EPS)
    mu_c = np.repeat(mu_g, C // G, axis=1)      # [B, C]
    rstd_c = np.repeat(rstd_g, C // G, axis=1)  # [B, C]

    scale_vec = np.concatenate(
        [np.full(128, s1 * ALPHA), np.full(128, s1 / ALPHA), np.ones(128)]
    ).astype(np.float32)

    in_maps = []
    for core in range(NCORES):
        b, h = divmod(core, NH)
        rows = w_qkv[384 * h : 384 * (h + 1)]          # (384, 512)
        wfold = rows * norm_w[None, :] * scale_vec[:, None]
        bias0 = (rows @ norm_b) * scale_vec
        wpp = wfold * rstd_c[b][None, :]               # W'' with rstd folded
        bias_full = bias0 - wpp @ mu_c[b]
        wq8 = np.ascontiguousarray(
            wpp.T.reshape(2, 2, 128, 384).transpose(0, 2, 1, 3).astype(f8)
        )
        bqkv = np.ascontiguousarray(
            bias_full[:256].reshape(2, 128).T.astype(np.float32)
        )
        wprojT = np.ascontiguousarray(
            w_proj[:, 128 * h : 128 * (h + 1)].T.astype(bf16)
        )
        xb = x[b].reshape(C, N)
        x2 = np.ascontiguousarray(
            xb.reshape(2, 2, 128, N).transpose(0, 2, 1, 3).astype(f8)
        )
        in_maps.append(
            {
                "x2": x2,
                "wq8": wq8,
                "bqkv": bqkv,
                "wprojT": wprojT,
                "ident": np.eye(128, dtype=bf16),
            }
        )

    nc = _get_nc()
    res = run_bass_kernel_spmd(
        nc,
        in_maps,
        list(range(NCORES)),
        trace=TRACE,
        trace_cores=TRACE_CORES if TRACE else None,
    )
    LAST_RESULT = res

    out = np.empty((B, C, N), dtype=np.float32)
    for b in range(B):
        acc = x[b].reshape(C, N) + b_proj[:, None]
        for h in range(NH):
            r = res.results[4 * b + h]
            acc = acc + r["partial"].astype(np.float32) / r["zout"]
            # v-bias compensation: attention rows sum to 1
            rows_v = w_qkv[384 * h + 256 : 384 * (h + 1)]
            wv_fold = rows_v * norm_w[None, :]
            bias_v = rows_v @ norm_b - (wv_fold * rstd_c[b]) @ mu_c[b]
            acc = acc + (w_proj[:, 128 * h : 128 * (h + 1)] @ bias_v)[:, None]
        out[b] = acc
    return out.reshape(B, C, 64, 64)


# revision 25
# speedup vs baseline: 1.1734x; 1.1734x over previous
"""AttentionBlock (GroupNorm -> qkv conv1x1 -> 4-head attention -> proj + residual)
on 8 Trainium2 NeuronCores.

Sharding: B*NH = 2*4 = 8 (batch, head) pairs -> one per core.

Host folds everything input-dependent that is O(C*N): GroupNorm stats
(mu, rstd per group), gamma/beta, qk scale s1 and the alpha q/k range
split are all folded into the shipped fp8 weights W'' and exact f32
biases.  x ships as fp8e4 in kt-pair layout.

Per core (head h of batch b):
  - qkv GEMM in fp8 DoubleRow (contraction 256/pass), bias added during
    the PSUM->SBUF copies on DVE; v transposed on the PE (identity
    matmul) and packed to fp8 for DoubleRow.
  - scoresT[s,t] = k_tile.T @ q (bf16, s on partitions)
  - exp on ACT only: fp8e4 out with +2.8 offset (cancels in partial/Z)
  - attn@v: fp8 DoubleRow over s-tile pairs
  - Z[t]: most pairs summed by DoubleRow ones-matmuls on PE, the rest by
    a DVE fp16 add tree joined into the same PSUM group (round 3 uses
    the earliest pairs for the tree so the drain is not gated on it).
  - proj on PE (bf16); partial shipped bf16; v-bias compensated on host
    (attention rows sum to 1, so it shifts h/Z by exactly bias_v).
Host: out[b] = x + b_proj + sum_h (partial_h/Z_h + Wp_h @ bias_v_h).
"""

import math
from contextlib import ExitStack

import ml_dtypes
import numpy as np

import concourse.bacc as bacc
import concourse.bass as bass
import concourse.mybir as mybir
import concourse.tile as tile
from concourse.bass_utils import run_bass_kernel_spmd

C = 512
NH = 4
G = 32
EPS = 1e-5
N = 4096
CH = 128
B = 2
NCORES = 8
TCHUNK = 1024
NCHUNK = N // TCHUNK     # 4
NST = N // 128           # 32
NPAIR = NST // 2         # 16
N_DVE_Z = 6              # s-tile pairs of Z summed on the DVE tree
EXPB = 2.8               # exp offset: e^(score+2.8) stays within fp8e4 range
ALPHA = 4.0              # q-row fold (k rows get 1/ALPHA); cancels in q.k

F32 = mybir.dt.float32
BF16 = mybir.dt.bfloat16
F16 = mybir.dt.float16
F8 = mybir.dt.float8e4
DR = mybir.MatmulPerfMode.DoubleRow
AF = mybir.ActivationFunctionType

TRACE = False
TRACE_CORES = [0]
LAST_RESULT = None


def build_program():
    nc = bacc.Bacc()

    # x2[pr, p, i, n] = x[128*(2pr+i)+p, n]  (kt-pair layout for DoubleRow)
    x2d = nc.declare_dram_parameter("x2", [2, 128, 2, N], F8, isOutput=False)
    # wq8[pr, p, i, o] = W''[o, 128*(2pr+i)+p]  (all folds + rstd, fp8)
    wq8d = nc.declare_dram_parameter("wq8", [2, 128, 2, 384], F8, isOutput=False)
    bqkv = nc.declare_dram_parameter("bqkv", [128, 2], F32, isOutput=False)
    wprojT = nc.declare_dram_parameter("wprojT", [CH, C], BF16, isOutput=False)
    ident = nc.declare_dram_parameter("ident", [128, 128], BF16, isOutput=False)
    partial = nc.declare_dram_parameter("partial", [C, N], BF16, isOutput=True)
    zout = nc.declare_dram_parameter("zout", [1, N], F32, isOutput=True)

    with tile.TileContext(nc) as tc, ExitStack() as ctx:
        consts = ctx.enter_context(tc.tile_pool(name="consts", bufs=1))
        xpool = ctx.enter_context(tc.tile_pool(name="xpool", bufs=2))
        qkp = ctx.enter_context(tc.tile_pool(name="qkp", bufs=1))
        epool = ctx.enter_context(tc.tile_pool(name="epool", bufs=32))
        trpool = ctx.enter_context(tc.tile_pool(name="trpool", bufs=14))
        hpool = ctx.enter_context(tc.tile_pool(name="hpool", bufs=2))
        opool = ctx.enter_context(tc.tile_pool(name="opool", bufs=3))
        zsp = ctx.enter_context(tc.tile_pool(name="zsp", bufs=1))
        ps_sc = ctx.enter_context(tc.tile_pool(name="ps_sc", bufs=2, space="PSUM"))
        ps_acc = ctx.enter_context(tc.tile_pool(name="ps_acc", bufs=2, space="PSUM"))
        ps_mm2 = ctx.enter_context(tc.tile_pool(name="ps_mm2", bufs=2, space="PSUM"))

        # ---- x first (3 queues, chunk-0-relevant halves first), then weights
        xt = [xpool.tile([128, 2, N], F8, tag="xt", name=f"x{pr}")
              for pr in range(2)]
        H2 = N // 2
        w8 = []
        for pr in range(2):
            w8.append(consts.tile([128, 2, 384], F8, tag=f"w8{pr}", name=f"w8{pr}"))
        # x streamed as kt-halves: the low halves (cols 0:2048) land first so
        # the k/q chunk GEMMs and round-0 scores can start while the rest loads
        qs = [nc.sync, nc.scalar, nc.scalar, nc.sync]
        for hv in range(2):
            for kt in range(4):
                pr, i = divmod(kt, 2)
                qs[kt].dma_start(
                    out=xt[pr][:, i, hv * H2 : (hv + 1) * H2],
                    in_=x2d[pr, :, i, hv * H2 : (hv + 1) * H2],
                )
            if hv == 0:
                for pr in range(2):
                    nc.scalar.dma_start(out=w8[pr], in_=wq8d[pr])
        bq_sb = consts.tile([128, 2], F32, tag="bq")
        nc.gpsimd.dma_start(out=bq_sb, in_=bqkv[:, :])
        wp_sb = consts.tile([CH, C], BF16, tag="wp")
        nc.gpsimd.dma_start(out=wp_sb, in_=wprojT[:, :])
        ident_sb = consts.tile([128, 128], BF16, tag="ident")
        nc.gpsimd.dma_start(out=ident_sb, in_=ident[:, :])
        ones2 = consts.tile([128, 2, 16], F8, tag="ones2")
        nc.vector.memset(ones2, 1.0)
        ones1 = consts.tile([128, 1], F16, tag="ones1")
        nc.vector.memset(ones1, 1.0)
        bexp = consts.tile([128, 1], F32, tag="bexp")
        nc.vector.memset(bexp, EXPB)

        # ---- qkv chunk GEMM (DoubleRow) ----
        q_sb = qkp.tile([128, N], BF16, tag="q_sb")
        k_sb = qkp.tile([128, N], BF16, tag="k_sb")
        v_sb = qkp.tile([128, N], BF16, tag="v_sb")
        vT2 = qkp.tile([128, NST, 128], F8, tag="vT2")

        def qkv_chunk(j, ch):
            ps = ps_mm2.tile([128, 512], F32, tag="mm2", name=f"qkv{j}_{ch}")
            for pr in range(2):
                nc.tensor.matmul(
                    ps,
                    lhsT=w8[pr][:, :, 128 * j : 128 * (j + 1)],
                    rhs=xt[pr][:, :, 512 * ch : 512 * (ch + 1)],
                    start=(pr == 0),
                    stop=(pr == 1),
                    perf_mode=DR,
                )
            dst = (q_sb, k_sb, v_sb)[j]
            if j < 2:
                nc.vector.tensor_scalar(
                    out=dst[:, 512 * ch : 512 * (ch + 1)],
                    in0=ps,
                    scalar1=bq_sb[:, j : j + 1],
                    scalar2=None,
                    op0=mybir.AluOpType.add,
                )
            else:
                nc.vector.tensor_copy(
                    out=dst[:, 512 * ch : 512 * (ch + 1)], in_=ps
                )
                # transpose the 4 s-tiles of this chunk on the PE, then pack
                # bf16 psum -> fp8 vT2 in one DVE copy
                ps_tr = ps_acc.tile([128, 4, 128], BF16, tag="acc",
                                    name=f"tr{ch}")
                for l in range(4):
                    stt = 4 * ch + l
                    nc.tensor.transpose(
                        ps_tr[:, l, :],
                        v_sb[:, 128 * stt : 128 * (stt + 1)],
                        ident_sb,
                    )
                nc.vector.tensor_copy(
                    out=vT2[:, 4 * ch : 4 * (ch + 1), :], in_=ps_tr
                )

        # ---- round machinery ----
        et_all = []          # per round: list of NPAIR pair tiles
        esums = [None] * NCHUNK
        ph_all = [None] * NCHUNK

        def tree_pairs(r):
            # last round: tree over the EARLY pairs (their exps land first)
            # and more of them, so the drain isn't gated on PE z-matmuls
            return range(0, 10) if r == NCHUNK - 1 else \
                range(NPAIR - N_DVE_Z, NPAIR)

        def pe_pairs(r):
            return [p for p in range(NPAIR) if p not in set(tree_pairs(r))]

        def emit_scores(r, stt):
            ps = ps_sc.tile([128, TCHUNK], F32, tag="sc")
            ksl = k_sb[:, 128 * stt : 128 * (stt + 1)]
            t0 = r * TCHUNK
            for hh in range(2):
                nc.tensor.matmul(
                    ps[:, 512 * hh : 512 * (hh + 1)],
                    lhsT=ksl,
                    rhs=q_sb[:, t0 + 512 * hh : t0 + 512 * (hh + 1)],
                    start=True,
                    stop=True,
                )
            if stt % 2 == 0:
                et = epool.tile([128, 2, TCHUNK], F8, tag="et")
                et_all[r].append(et)
            nc.scalar.activation(
                out=et_all[r][stt // 2][:, stt % 2, :],
                in_=ps,
                func=AF.Exp,
                bias=bexp,
            )

        def emit_attnv_pair(r, p):
            ph = ph_all[r]
            for hh in range(2):
                nc.tensor.matmul(
                    ph[hh],
                    lhsT=vT2[:, 2 * p : 2 * p + 2, :],
                    rhs=et_all[r][p][:, :, 512 * hh : 512 * (hh + 1)],
                    start=(p == 0),
                    stop=(p == NPAIR - 1),
                    perf_mode=DR,
                )

        def emit_z(r):
            # both t-halves; PE pairs + fp16 esum join
            for hh in range(2):
                zps = ps_mm2.tile([1, 512], F32, tag="mm2", name=f"z{r}_{hh}")
                pep = pe_pairs(r)
                for n_, p in enumerate(pep):
                    nc.tensor.matmul(
                        zps,
                        lhsT=ones2[:, :, 0:1],
                        rhs=et_all[r][p][:, :, 512 * hh : 512 * (hh + 1)],
                        start=(n_ == 0),
                        stop=False,
                        perf_mode=DR,
                    )
                nc.tensor.matmul(
                    zps,
                    lhsT=ones1,
                    rhs=esums[r][:, 512 * hh : 512 * (hh + 1)],
                    start=False,
                    stop=True,
                )
                nc.vector.tensor_copy(
                    out=z_sb[:, r * TCHUNK + 512 * hh : r * TCHUNK + 512 * (hh + 1)],
                    in_=zps,
                )

        def emit_tree(r):
            tt = []
            for p in tree_pairs(r):
                t_ = trpool.tile([128, TCHUNK], F16, tag="tr", name=f"t{r}_{p}")
                nc.vector.tensor_add(
                    out=t_, in0=et_all[r][p][:, 0, :], in1=et_all[r][p][:, 1, :]
                )
                tt.append(t_)
            while len(tt) > 1:
                nxt = []
                for a in range(0, len(tt) - 1, 2):
                    nc.vector.tensor_add(out=tt[a], in0=tt[a], in1=tt[a + 1])
                    nxt.append(tt[a])
                if len(tt) % 2 == 1:
                    nxt.append(tt[-1])
                tt = nxt
            esums[r] = tt[0]

        def emit_hcopy_proj(r):
            hsb = hpool.tile([128, TCHUNK], BF16, tag="h")
            for hh in range(2):
                nc.vector.tensor_copy(
                    out=hsb[:, 512 * hh : 512 * (hh + 1)], in_=ph_all[r][hh]
                )
            for ot in range(4):
                for hh in range(2):
                    psp = ps_mm2.tile([128, 512], F32, tag="mm2",
                                      name=f"pj{r}_{ot}_{hh}")
                    nc.tensor.matmul(
                        psp,
                        lhsT=wp_sb[:, 128 * ot : 128 * (ot + 1)],
                        rhs=hsb[:, 512 * hh : 512 * (hh + 1)],
                        start=True,
                        stop=True,
                    )
                    ob = opool.tile([128, 512], BF16, tag="ob")
                    nc.vector.tensor_copy(out=ob, in_=psp)
                    nc.sync.dma_start(
                        out=partial[
                            128 * ot : 128 * (ot + 1),
                            r * TCHUNK + 512 * hh : r * TCHUNK + 512 * (hh + 1),
                        ],
                        in_=ob,
                    )

        z_sb = zsp.tile([1, N], F32, tag="z_sb")

        # ================= round 0 =================
        et_all.append([])
        qkv_chunk(1, 0)
        qkv_chunk(0, 0)
        qkv_chunk(0, 1)
        emit_scores(0, 0)
        emit_scores(0, 1)
        fill0 = [("k", ch) for ch in range(1, 8)]
        extra0 = [("v", ch) for ch in range(8)] + [("q", 2), ("q", 3)]
        ei = 0
        for stt in range(2, NST):
            ch = stt // 4 + 1
            if fill0 and stt % 4 == 2 and ch < 8:
                qkv_chunk(1, ch)
                fill0.pop(0)
            if stt % 2 == 1 and ei < len(extra0):
                kind, ch2 = extra0[ei]
                qkv_chunk(2 if kind == "v" else 0, ch2)
                ei += 1
            emit_scores(0, stt)
        while ei < len(extra0):
            kind, ch2 = extra0[ei]
            qkv_chunk(2 if kind == "v" else 0, ch2)
            ei += 1
        emit_tree(0)

        # ================= rounds 1..3 =================
        ap3 = 0
        for r in range(1, NCHUNK):
            last = r == NCHUNK - 1
            et_all.append([])
            ph_all[r - 1] = [
                ps_acc.tile([128, 512], F32, tag="acc", name=f"h{r-1}_{hh}")
                for hh in range(2)
            ]
            if last:
                ph_all[r] = [
                    ps_acc.tile([128, 512], F32, tag="acc", name=f"h{r}_{hh}")
                    for hh in range(2)
                ]
            if r >= 2:
                emit_hcopy_proj(r - 2)
            pend_q = [("q", 4), ("q", 5), ("q", 6), ("q", 7)] if r == 1 else []
            ap = 0
            denom = 20 if last else NST
            for stt in range(NST):
                emit_scores(r, stt)
                if pend_q and stt % 2 == 1:
                    qkv_chunk(0, pend_q.pop(0)[1])
                if stt == 10:
                    emit_z(r - 1)
                want = min(NPAIR, ((stt + 1) * NPAIR) // denom)
                while ap < want:
                    emit_attnv_pair(r - 1, ap)
                    ap += 1
                if last:
                    # ph[3] reuses ph[2]'s psum slots: attnv(3) may only
                    # start after the h(2) copies, or the PE queue deadlocks
                    if stt == 20:
                        emit_hcopy_proj(r - 1)
                    while ap3 < min(max(0, stt - 21), NPAIR - 1):
                        emit_attnv_pair(r, ap3)
                        ap3 += 1
            while ap < NPAIR:
                emit_attnv_pair(r - 1, ap)
                ap += 1
            emit_tree(r)

        # ================= drain =================
        r = NCHUNK - 1
        while ap3 < NPAIR:
            emit_attnv_pair(r, ap3)
            ap3 += 1
        emit_z(r)
        emit_hcopy_proj(r)
        nc.sync.dma_start(out=zout[:, :], in_=z_sb)

    if not nc.is_finalized():
        nc.finalize()
    return nc


_NC_CACHE = None


def _get_nc():
    global _NC_CACHE
    if _NC_CACHE is None:
        _NC_CACHE = build_program()
    return _NC_CACHE


def kernel(x, norm_w, norm_b, w_qkv, w_proj, b_proj):
    global LAST_RESULT
    x = np.asarray(x, dtype=np.float32)
    norm_w = np.asarray(norm_w, dtype=np.float32)
    norm_b = np.asarray(norm_b, dtype=np.float32)
    w_qkv = np.asarray(w_qkv, dtype=np.float32)
    w_proj = np.asarray(w_proj, dtype=np.float32)
    b_proj = np.asarray(b_proj, dtype=np.float32)

    s1 = 1.0 / math.sqrt(math.sqrt(CH))
    bf16 = ml_dtypes.bfloat16
    f8 = ml_dtypes.float8_e4m3

    # host-side GroupNorm stats (folded into W''/biases; O(C*N) prep)
    xr = x.reshape(B, G, C // G * N)
    mu_g = xr.mean(axis=2)
    var_g = xr.var(axis=2)
    rstd_g = 1.0 / np.sqrt(var_g + EPS)
    mu_c = np.repeat(mu_g, C // G, axis=1)      # [B, C]
    rstd_c = np.repeat(rstd_g, C // G, axis=1)  # [B, C]

    scale_vec = np.concatenate(
        [np.full(128, s1 * ALPHA), np.full(128, s1 / ALPHA), np.ones(128)]
    ).astype(np.float32)

    in_maps = []
    for core in range(NCORES):
        b, h = divmod(core, NH)
        rows = w_qkv[384 * h : 384 * (h + 1)]          # (384, 512)
        wfold = rows * norm_w[None, :] * scale_vec[:, None]
        bias0 = (rows @ norm_b) * scale_vec
        wpp = wfold * rstd_c[b][None, :]               # W'' with rstd folded
        bias_full = bias0 - wpp @ mu_c[b]
        wq8 = np.ascontiguousarray(
            wpp.T.reshape(2, 2, 128, 384).transpose(0, 2, 1, 3).astype(f8)
        )
        bqkv = np.ascontiguousarray(
            bias_full[:256].reshape(2, 128).T.astype(np.float32)
        )
        wprojT = np.ascontiguousarray(
            w_proj[:, 128 * h : 128 * (h + 1)].T.astype(bf16)
        )
        xb = x[b].reshape(C, N)
        x2 = np.ascontiguousarray(
            xb.reshape(2, 2, 128, N).transpose(0, 2, 1, 3).astype(f8)
        )
        in_maps.append(
            {
                "x2": x2,
                "wq8": wq8,
                "bqkv": bqkv,
                "wprojT": wprojT,
                "ident": np.eye(128, dtype=bf16),
            }
        )

    nc = _get_nc()
    res = run_bass_kernel_spmd(
        nc,
        in_maps,
        list(range(NCORES)),
        trace=TRACE,
        trace_cores=TRACE_CORES if TRACE else None,
    )
    LAST_RESULT = res

    out = np.empty((B, C, N), dtype=np.float32)
    for b in range(B):
        acc = x[b].reshape(C, N) + b_proj[:, None]
        for h in range(NH):
            r = res.results[4 * b + h]
            acc = acc + r["partial"].astype(np.float32) / r["zout"]
            # v-bias compensation: attention rows sum to 1
            rows_v = w_qkv[384 * h + 256 : 384 * (h + 1)]
            wv_fold = rows_v * norm_w[None, :]
            bias_v = rows_v @ norm_b - (wv_fold * rstd_c[b]) @ mu_c[b]
            acc = acc + (w_proj[:, 128 * h : 128 * (h + 1)] @ bias_v)[:, None]
        out[b] = acc
    return out.reshape(B, C, 64, 64)
